# revision 28
# baseline (speedup 1.0000x reference)
"""Trainium2 Bass kernel for ChemiNet-style NNConv GNN (8 NeuronCores).

Math restructure: the final output per graph g collapses to
    out[g] = sum_{e: dst in g} m1[e] + sum_{n in g} hx[n] + cnt_g*cb1 + ob
with
    m1[e] = sum_{i,o} ow[o] * x[src_e,i] * relu(q[e, i, o])
    q[e]  = edge_attr_aug[e] @ Wfold          (bias folded as 13th input row)
    hx[n] = x[n] . (root_w @ ow)
    cb1   = conv_b . ow,  ob = out_b[0]
|ow_o| is folded into Wfold's columns so only sign(ow_o) remains downstream.

Sharding: graphs are split into 8 contiguous ranges balanced by edge count
(batch is sorted, so node ranges are contiguous); each edge is routed to the
core owning its destination graph; the host gathers x[src] rows per edge
while permuting edges (pure data movement), and reduces the per-edge /
per-node device scalars into per-graph sums with one bincount at the end.

Device pipelines (version-selectable):
  v0 (flat): PE matmul q=[128 edges, 750] -> one fused DVE
     scalar_tensor_tensor per sign-block: (q max 0) * x_bcast with
     accum_out = per-edge scalar. Simple, DVE-bound.
  v1/v2 (T-layout): PE computes q chunks [128 io-rows, e]; ScalarE/VectorE
     relu-evacuate PSUM->SBUF bf16 (split by chunk); PE multiplies by a
     constant signed selector [128,75] reducing over o into r=[75, e];
     VectorE multiplies by x^T; PE ones-dot reduces over i to per-edge
     scalars. v2 uses 1024-wide elementwise ops (halves DVE drain count)
     and runs the node-term dot on PE as well.
"""

import numpy as np

F_ATOM = 75
F_BOND = 12
OUT = 10
G_TOTAL = 2048
N_CORES = 8
P = 128           # partitions
ST = 16           # tiles per DMA/output group
EDGE_GRAN = P * ST
NODE_GRAN = P * ST

_PROG_CACHE = {}

GROUP = 512            # edges per matmul group (v1)
BATCH = 4              # groups per batch (v1)
COLS_PAD = 768         # 750 padded to 6*128


def _build_program_v1(e_batches, n_tiles, act_relu_chunks, repeat=1):
    """T-layout program: PE computes q_T chunks [128 io, 512 e], ACT/DVE
    relu-evacuate to SBUF bf16, PE selector-matmuls reduce over o with signs
    into r [75, e], DVE multiplies by x_T, PE ones-dot yields per-edge
    scalars."""
    import concourse.bacc as bacc
    import concourse.mybir as mybir
    import concourse.tile as tile

    f32 = mybir.dt.float32
    bf16 = mybir.dt.bfloat16
    nc = bacc.Bacc(None, target_bir_lowering=False)

    EPC = e_batches * BATCH * GROUP
    NPC = n_tiles * P
    eaT = nc.declare_dram_parameter("eaT", [F_BOND + 1, EPC], bf16, isOutput=False)
    xT = nc.declare_dram_parameter("xT", [F_ATOM, EPC], bf16, isOutput=False)
    Wf = nc.declare_dram_parameter("Wf", [F_BOND + 1, COLS_PAD], bf16, isOutput=False)
    Ssel = nc.declare_dram_parameter("Ssel", [P, 6 * F_ATOM], bf16, isOutput=False)
    ones = nc.declare_dram_parameter("ones", [F_ATOM, 1], bf16, isOutput=False)
    xs = nc.declare_dram_parameter("xs", [NPC, F_ATOM], f32, isOutput=False)
    rw1 = nc.declare_dram_parameter("rw1", [P, F_ATOM], f32, isOutput=False)
    m1o = nc.declare_dram_parameter("m1o", [e_batches, 1, BATCH * GROUP], f32, isOutput=True)
    hxo = nc.declare_dram_parameter("hxo", [n_tiles // ST, P, ST], f32, isOutput=True)

    mul = mybir.AluOpType.mult
    add = mybir.AluOpType.add

    with tile.TileContext(nc) as tc:
        with (
            tc.tile_pool(name="const", bufs=1) as cp,
            tc.tile_pool(name="ea", bufs=3) as eap,
            tc.tile_pool(name="xtp", bufs=3) as xtp,
            tc.tile_pool(name="qps", bufs=3, space="PSUM") as qpool,
            tc.tile_pool(name="rps", bufs=3, space="PSUM") as rpool,
            tc.tile_pool(name="mps", bufs=2, space="PSUM") as mpool,
            tc.tile_pool(name="rq", bufs=48) as rqpool,
            tc.tile_pool(name="u", bufs=3) as upool,
            tc.tile_pool(name="m1r", bufs=2) as m1rp,
            tc.tile_pool(name="xnp", bufs=3) as xnp,
            tc.tile_pool(name="scr", bufs=2) as scrp,
            tc.tile_pool(name="strip", bufs=2) as stp,
        ):
            Wt = cp.tile([F_BOND + 1, COLS_PAD], bf16)
            nc.sync.dma_start(Wt[:], Wf[:])
            St = cp.tile([P, 6 * F_ATOM], bf16)
            nc.sync.dma_start(St[:], Ssel[:])
            ot = cp.tile([F_ATOM, 1], bf16)
            nc.sync.dma_start(ot[:], ones[:])
            rt = cp.tile([P, F_ATOM], f32)
            nc.sync.dma_start(rt[:], rw1[:])

            def _phases():
                # ---- node phase ----
                for g in range(n_tiles // ST):
                    hxs = stp.tile([P, ST], f32, tag="hxs")
                    xsl = xnp.tile([P, ST * F_ATOM], f32, tag="xsl")
                    src = xs[g * NODE_GRAN:(g + 1) * NODE_GRAN, :].rearrange(
                        "(t p) f -> p t f", p=P)
                    nc.sync.dma_start(
                        xsl[:].rearrange("p (t f) -> p t f", f=F_ATOM), src)
                    for c in range(ST):
                        so = scrp.tile([P, F_ATOM], f32, tag="so")
                        nc.vector.scalar_tensor_tensor(
                            out=so[:], in0=xsl[:, c * F_ATOM:(c + 1) * F_ATOM],
                            scalar=1.0, in1=rt[:], op0=mul, op1=mul,
                            accum_out=hxs[:, c:c + 1])
                    nc.sync.dma_start(hxo[g], hxs[:])

                # ---- edge phase ----
                BE = BATCH * GROUP
                for b in range(e_batches):
                    esl = eap.tile([F_BOND + 1, BE], bf16, tag="esl")
                    nc.sync.dma_start(esl[:], eaT[:, b * BE:(b + 1) * BE])
                    xsl2 = xtp.tile([F_ATOM, BE], bf16, tag="xsl2")
                    nc.sync.dma_start(xsl2[:], xT[:, b * BE:(b + 1) * BE])
                    rq = {}
                    for c in range(6):
                        for g in range(BATCH):
                            q_ps = qpool.tile([P, GROUP], f32, tag="q")
                            nc.tensor.matmul(
                                q_ps[:], Wt[:, c * P:(c + 1) * P],
                                esl[:, g * GROUP:(g + 1) * GROUP],
                                start=True, stop=True)
                            t = rqpool.tile([P, GROUP], bf16, tag="rq")
                            if c in act_relu_chunks:
                                nc.scalar.activation(
                                    t[:], q_ps[:],
                                    mybir.ActivationFunctionType.Relu)
                            else:
                                nc.vector.tensor_scalar_max(t[:], q_ps[:], 0.0)
                            rq[(c, g)] = t
                    m1row = m1rp.tile([1, BE], f32, tag="m1row")
                    for g in range(BATCH):
                        r_ps = rpool.tile([F_ATOM, GROUP], f32, tag="r")
                        for c in range(6):
                            nc.tensor.matmul(
                                r_ps[:], St[:, c * F_ATOM:(c + 1) * F_ATOM],
                                rq[(c, g)][:], start=(c == 0), stop=(c == 5))
                        u = upool.tile([F_ATOM, GROUP], bf16, tag="u")
                        nc.vector.tensor_tensor(
                            out=u[:], in0=r_ps[:],
                            in1=xsl2[:, g * GROUP:(g + 1) * GROUP], op=mul)
                        m1_ps = mpool.tile([1, GROUP], f32, tag="m1ps")
                        nc.tensor.matmul(m1_ps[:], ot[:], u[:],
                                         start=True, stop=True)
                        nc.scalar.copy(m1row[0:1, g * GROUP:(g + 1) * GROUP],
                                       m1_ps[:])
                    nc.sync.dma_start(m1o[b], m1row[0:1, :])

            if repeat > 1:
                with tc.For_i(0, repeat, 1):
                    _phases()
            else:
                _phases()
    nc.compile()
    return nc


def _build_program_v2(e_batches, n_tiles, act_chunks, repeat=1):
    """v1 with drain-aware op sizing: GROUP=1024 (bf16 rhs), half the
    DVE/ACT op count, node phase on PE (rw1-dot), m1/hx PSUM tiles share the
    r-pool slots to stay within 8 banks."""
    import concourse.bacc as bacc
    import concourse.mybir as mybir
    import concourse.tile as tile

    f32 = mybir.dt.float32
    bf16 = mybir.dt.bfloat16
    nc = bacc.Bacc(None, target_bir_lowering=False)

    G2 = 1024
    NG = 2                      # groups per batch (2048 edges)
    BE = G2 * NG
    EPC = e_batches * BE
    NPC = n_tiles * P
    NCH = NPC // G2
    eaT = nc.declare_dram_parameter("eaT", [F_BOND + 1, EPC], bf16, isOutput=False)
    xT = nc.declare_dram_parameter("xT", [F_ATOM, EPC], bf16, isOutput=False)
    Wf = nc.declare_dram_parameter("Wf", [F_BOND + 1, COLS_PAD], bf16, isOutput=False)
    Ssel = nc.declare_dram_parameter("Ssel", [P, 6 * F_ATOM], bf16, isOutput=False)
    ones = nc.declare_dram_parameter("ones", [F_ATOM, 1], bf16, isOutput=False)
    xsT = nc.declare_dram_parameter("xsT", [F_ATOM, NPC], bf16, isOutput=False)
    rw1 = nc.declare_dram_parameter("rw1", [F_ATOM, 1], bf16, isOutput=False)
    m1o = nc.declare_dram_parameter("m1o", [e_batches, 1, BE], f32, isOutput=True)
    hxo = nc.declare_dram_parameter("hxo", [1, NPC], f32, isOutput=True)

    mul = mybir.AluOpType.mult

    with tile.TileContext(nc) as tc:
        with (
            tc.tile_pool(name="const", bufs=1) as cp,
            tc.tile_pool(name="ea", bufs=3) as eap,
            tc.tile_pool(name="xtp", bufs=3) as xtp,
            tc.tile_pool(name="qps", bufs=2, space="PSUM") as qpool,
            tc.tile_pool(name="rps", bufs=2, space="PSUM") as rpool,
            tc.tile_pool(name="rq", bufs=36) as rqpool,
            tc.tile_pool(name="u", bufs=4) as upool,
            tc.tile_pool(name="m1r", bufs=2) as m1rp,
            tc.tile_pool(name="hxr", bufs=1) as hxrp,
        ):
            Wt = cp.tile([F_BOND + 1, COLS_PAD], bf16)
            nc.sync.dma_start(Wt[:], Wf[:])
            St = cp.tile([P, 6 * F_ATOM], bf16)
            nc.sync.dma_start(St[:], Ssel[:])
            ot = cp.tile([F_ATOM, 1], bf16)
            nc.sync.dma_start(ot[:], ones[:])
            rt = cp.tile([F_ATOM, 1], bf16)
            nc.sync.dma_start(rt[:], rw1[:])
            xst = cp.tile([F_ATOM, NPC], bf16)
            nc.sync.dma_start(xst[:], xsT[:])

            def _phases():
                # ---- node phase: hx = rw1 . x, via PE ----
                hxrow = hxrp.tile([1, NPC], f32, tag="hxrow")
                for k in range(NCH):
                    hx_ps = rpool.tile([1, G2], f32, tag="r")
                    for h in range(2):
                        nc.tensor.matmul(
                            hx_ps[:, h * 512:(h + 1) * 512], rt[:],
                            xst[:, k * G2 + h * 512:k * G2 + (h + 1) * 512],
                            start=True, stop=True)
                    nc.scalar.copy(hxrow[0:1, k * G2:(k + 1) * G2], hx_ps[:])
                nc.sync.dma_start(hxo[0:1, :], hxrow[:])

                # ---- edge phase ----
                for b in range(e_batches):
                    esl = eap.tile([F_BOND + 1, BE], bf16, tag="esl")
                    nc.sync.dma_start(esl[:], eaT[:, b * BE:(b + 1) * BE])
                    xsl2 = xtp.tile([F_ATOM, BE], bf16, tag="xsl2")
                    nc.sync.dma_start(xsl2[:], xT[:, b * BE:(b + 1) * BE])
                    rq = {}
                    for c in range(6):
                        for g in range(NG):
                            q_ps = qpool.tile([P, G2], f32, tag="q")
                            for h in range(2):
                                nc.tensor.matmul(
                                    q_ps[:, h * 512:(h + 1) * 512],
                                    Wt[:, c * P:(c + 1) * P],
                                    esl[:, g * G2 + h * 512:
                                         g * G2 + (h + 1) * 512],
                                    start=True, stop=True)
                            t = rqpool.tile([P, G2], bf16, tag="rq")
                            if c in act_chunks:
                                nc.scalar.activation(
                                    t[:], q_ps[:],
                                    mybir.ActivationFunctionType.Relu)
                            else:
                                nc.vector.tensor_scalar_max(t[:], q_ps[:], 0.0)
                            rq[(c, g)] = t
                    m1row = m1rp.tile([1, BE], f32, tag="m1row")
                    for g in range(NG):
                        r_ps = rpool.tile([F_ATOM, G2], f32, tag="r")
                        for c in range(6):
                            for h in range(2):
                                nc.tensor.matmul(
                                    r_ps[:, h * 512:(h + 1) * 512],
                                    St[:, c * F_ATOM:(c + 1) * F_ATOM],
                                    rq[(c, g)][:, h * 512:(h + 1) * 512],
                                    start=(c == 0), stop=(c == 5))
                        u = upool.tile([F_ATOM, G2], bf16, tag="u")
                        nc.vector.tensor_tensor(
                            out=u[:], in0=r_ps[:],
                            in1=xsl2[:, g * G2:(g + 1) * G2], op=mul)
                        m1_ps = rpool.tile([1, G2], f32, tag="r")
                        for h in range(2):
                            nc.tensor.matmul(
                                m1_ps[:, h * 512:(h + 1) * 512], ot[:],
                                u[:, h * 512:(h + 1) * 512],
                                start=True, stop=True)
                        nc.scalar.copy(m1row[0:1, g * G2:(g + 1) * G2],
                                       m1_ps[:])
                    nc.sync.dma_start(m1o[b], m1row[0:1, :])

            if repeat > 1:
                with tc.For_i(0, repeat, 1):
                    _phases()
            else:
                _phases()
    nc.compile()
    return nc


def _build_program(e_tiles, n_tiles, kp, kn, repeat=1):
    import concourse.bacc as bacc
    import concourse.mybir as mybir
    import concourse.tile as tile

    f32 = mybir.dt.float32
    nc = bacc.Bacc(None, target_bir_lowering=False)

    EPC = e_tiles * P
    NPC = n_tiles * P
    eaT = nc.declare_dram_parameter("eaT", [F_BOND + 1, EPC], f32, isOutput=False)
    xe = nc.declare_dram_parameter("xe", [EPC, 2 * F_ATOM], f32, isOutput=False)
    xs = nc.declare_dram_parameter("xs", [NPC, F_ATOM], f32, isOutput=False)
    Wf = nc.declare_dram_parameter("Wf", [F_BOND + 1, F_ATOM * OUT], f32, isOutput=False)
    rw1 = nc.declare_dram_parameter("rw1", [P, F_ATOM], f32, isOutput=False)
    m1o = nc.declare_dram_parameter("m1o", [e_tiles // ST, P, ST], f32, isOutput=True)
    hxo = nc.declare_dram_parameter("hxo", [n_tiles // ST, P, ST], f32, isOutput=True)

    COLS = F_ATOM * OUT          # 750
    KPW = F_ATOM * kp            # width of positive block
    KNW = F_ATOM * kn

    mul = mybir.AluOpType.mult
    add = mybir.AluOpType.add
    mx = mybir.AluOpType.max

    with tile.TileContext(nc) as tc:
        with (
            tc.tile_pool(name="const", bufs=1) as cp,
            tc.tile_pool(name="ea", bufs=3) as eap,
            tc.tile_pool(name="xed", bufs=3) as xep,
            tc.tile_pool(name="ps", bufs=2, space="PSUM") as psp,
            tc.tile_pool(name="scr", bufs=2) as scrp,
            tc.tile_pool(name="strip", bufs=2) as stp,
            tc.tile_pool(name="acc", bufs=2) as accp,
        ):
            Wt = cp.tile([F_BOND + 1, COLS], f32)
            nc.sync.dma_start(Wt[:], Wf[:])
            rt = cp.tile([P, F_ATOM], f32)
            nc.sync.dma_start(rt[:], rw1[:])

            def _phases():
                # ---- node phase: hx[n] = x[n] . rw1 ----
                for g in range(n_tiles // ST):
                    hxs = stp.tile([P, ST], f32, tag="hxs")
                    xsl = xep.tile([P, ST * F_ATOM], f32, tag="xsl")
                    src = xs[g * NODE_GRAN:(g + 1) * NODE_GRAN, :].rearrange(
                        "(t p) f -> p t f", p=P)
                    nc.sync.dma_start(
                        xsl[:].rearrange("p (t f) -> p t f", f=F_ATOM), src)
                    for c in range(ST):
                        so = scrp.tile([P, F_ATOM], f32, tag="so")
                        nc.vector.scalar_tensor_tensor(
                            out=so[:], in0=xsl[:, c * F_ATOM:(c + 1) * F_ATOM],
                            scalar=1.0, in1=rt[:], op0=mul, op1=mul,
                            accum_out=hxs[:, c:c + 1])
                    nc.sync.dma_start(hxo[g], hxs[:])

                _edge_phase()

            def _edge_phase():
                for g in range(e_tiles // ST):
                    m1s = stp.tile([P, ST], f32, tag="m1s")
                    esl = eap.tile([F_BOND + 1, ST * P], f32, tag="esl")
                    nc.sync.dma_start(esl[:], eaT[:, g * EDGE_GRAN:(g + 1) * EDGE_GRAN])
                    xesl = xep.tile([P, ST * 2 * F_ATOM], f32, tag="xesl")
                    xsrc = xe[g * EDGE_GRAN:(g + 1) * EDGE_GRAN, :].rearrange(
                        "(t p) f -> p t f", p=P)
                    nc.sync.dma_start(
                        xesl[:].rearrange("p (t f) -> p t f", f=2 * F_ATOM), xsrc)
                    for c in range(ST):
                        ea_t = esl[:, c * P:(c + 1) * P]
                        xe_t = xesl[:, c * 2 * F_ATOM:(c + 1) * 2 * F_ATOM]
                        q = psp.tile([P, 768], f32, tag="q")
                        nc.tensor.matmul(q[:, 0:512], ea_t, Wt[:, 0:512],
                                         start=True, stop=True)
                        nc.tensor.matmul(q[:, 512:COLS], ea_t, Wt[:, 512:COLS],
                                         start=True, stop=True)
                        po = scrp.tile([P, COLS], f32, tag="po")
                        m1a = accp.tile([P, 1], f32, tag="m1a")
                        m1b = accp.tile([P, 1], f32, tag="m1b")
                        if kp > 0:
                            nc.vector.scalar_tensor_tensor(
                                out=po[:, 0:KPW].rearrange("p (i o) -> p i o", o=kp),
                                in0=q[:, 0:KPW].rearrange("p (i o) -> p i o", o=kp),
                                scalar=0.0,
                                in1=xe_t[:, 0:F_ATOM].broadcast_to([P, F_ATOM, kp]),
                                op0=mx,
                                op1=mul,
                                accum_out=m1a[:],
                            )
                        else:
                            nc.vector.memset(m1a[:], 0.0)
                        if kn > 0:
                            nc.vector.scalar_tensor_tensor(
                                out=po[:, KPW:COLS].rearrange("p (i o) -> p i o", o=kn),
                                in0=q[:, KPW:COLS].rearrange("p (i o) -> p i o", o=kn),
                                scalar=0.0,
                                in1=xe_t[:, F_ATOM:2 * F_ATOM]
                                    .broadcast_to([P, F_ATOM, kn]),
                                op0=mx,
                                op1=mul,
                                accum_out=m1b[:],
                            )
                        else:
                            nc.vector.memset(m1b[:], 0.0)
                        nc.scalar.add(m1s[:, c:c + 1], m1a[:], add=m1b[:])
                    nc.sync.dma_start(m1o[g], m1s[:])

            if repeat > 1:
                with tc.For_i(0, repeat, 1):
                    _phases()
            else:
                _phases()
    nc.compile()
    return nc


def _prep(x, edge_index, edge_attr, batch, lin_w, lin_b, root_w, conv_b,
          out_w, out_b, G):
    """Host-side sharding + weight folding. Returns per-core input maps and
    metadata for the final combine."""
    E = edge_index.shape[1]
    N = x.shape[0]

    src = edge_index[0].astype(np.int64)
    dst = edge_index[1].astype(np.int64)
    ge = batch[dst]                       # graph of each edge's destination
    perm = np.argsort(ge, kind="stable")
    ge_s = ge[perm]
    src_s = src[perm]
    ea_s = edge_attr[perm]

    ecnt = np.bincount(ge_s, minlength=G)
    ecum = np.concatenate([[0], np.cumsum(ecnt)])
    ncnt = np.bincount(batch, minlength=G)
    ncum = np.concatenate([[0], np.cumsum(ncnt)])

    # split graphs into N_CORES contiguous ranges, balanced by edge count
    gb = [0]
    for c in range(1, N_CORES):
        gb.append(int(np.searchsorted(ecum[1:], E * c / N_CORES)))
    gb.append(G)
    gb = np.array(gb)

    e_rngs = [(int(ecum[gb[c]]), int(ecum[gb[c + 1]])) for c in range(N_CORES)]
    n_rngs = [(int(ncum[gb[c]]), int(ncum[gb[c + 1]])) for c in range(N_CORES)]

    max_e = max(e1 - e0 for e0, e1 in e_rngs)
    max_n = max(n1 - n0 for n0, n1 in n_rngs)
    EPC = -(-max_e // EDGE_GRAN) * EDGE_GRAN
    NPC = -(-max_n // NODE_GRAN) * NODE_GRAN

    # weight folding: |ow| into rows, sign via column blocks, i-major o-minor
    ow = out_w.reshape(-1).astype(np.float64)
    o_pos = np.where(ow >= 0)[0]
    o_neg = np.where(ow < 0)[0]
    kp, kn = len(o_pos), len(o_neg)
    o_order = np.concatenate([o_pos, o_neg]).astype(np.int64)
    # column j of block: (i, o) i-major within each sign block
    i_idx = np.repeat(np.arange(F_ATOM), kp)
    o_idx = np.tile(o_pos, F_ATOM)
    rows_p = i_idx * OUT + o_idx
    i_idx = np.repeat(np.arange(F_ATOM), kn)
    o_idx = np.tile(o_neg, F_ATOM)
    rows_n = i_idx * OUT + o_idx
    rows = np.concatenate([rows_p, rows_n])
    absow = np.abs(ow)[np.concatenate([np.tile(o_pos, F_ATOM),
                                       np.tile(o_neg, F_ATOM)])]
    Wcols = lin_w[rows].astype(np.float64) * absow[:, None]          # [750,12]
    bcols = lin_b[rows].astype(np.float64) * absow                   # [750]
    Wf = np.concatenate([Wcols, bcols[:, None]], axis=1).T           # [13,750]
    Wf = np.ascontiguousarray(Wf, dtype=np.float32)

    rw1 = (root_w.astype(np.float64) @ ow).astype(np.float32)        # [75]
    rw1_rep = np.ascontiguousarray(np.broadcast_to(rw1[None, :], (P, F_ATOM)),
                                   dtype=np.float32)

    in_maps = []
    for c in range(N_CORES):
        e0, e1 = e_rngs[c]
        ne = e1 - e0
        eaT = np.zeros((F_BOND + 1, EPC), dtype=np.float32)
        eaT[:F_BOND, :ne] = ea_s[e0:e1].T
        eaT[F_BOND, :ne] = 1.0
        xsrc = x[src_s[e0:e1]].astype(np.float32)
        xef = np.zeros((EPC, 2 * F_ATOM), dtype=np.float32)
        xef[:ne, :F_ATOM] = xsrc
        xef[:ne, F_ATOM:] = -xsrc
        n0, n1 = n_rngs[c]
        nn = n1 - n0
        xsf = np.zeros((NPC, F_ATOM), dtype=np.float32)
        xsf[:nn] = x[n0:n1]
        in_maps.append({
            "eaT": eaT, "xe": xef, "xs": xsf, "Wf": Wf, "rw1": rw1_rep,
        })

    cb1 = float(np.dot(conv_b.astype(np.float64), ow))
    ob = float(np.asarray(out_b).reshape(-1)[0])
    meta = dict(gb=gb, e_rngs=e_rngs, n_rngs=n_rngs, ge_s=ge_s, batch=batch,
                ncnt=ncnt, cb1=cb1, ob=ob, EPC=EPC, NPC=NPC, kp=kp, kn=kn)
    return in_maps, meta


def _prep_v1(x, edge_index, edge_attr, batch, lin_w, lin_b, root_w, conv_b,
             out_w, out_b, G, v2=False):
    E = edge_index.shape[1]
    src = edge_index[0].astype(np.int64)
    dst = edge_index[1].astype(np.int64)
    ge = batch[dst]
    perm = np.argsort(ge, kind="stable")
    ge_s = ge[perm]
    src_s = src[perm]
    ea_s = edge_attr[perm]

    ecnt = np.bincount(ge_s, minlength=G)
    ecum = np.concatenate([[0], np.cumsum(ecnt)])
    ncnt = np.bincount(batch, minlength=G)
    ncum = np.concatenate([[0], np.cumsum(ncnt)])

    gb = [0]
    for c in range(1, N_CORES):
        gb.append(int(np.searchsorted(ecum[1:], E * c / N_CORES)))
    gb.append(G)
    gb = np.array(gb)

    e_rngs = [(int(ecum[gb[c]]), int(ecum[gb[c + 1]])) for c in range(N_CORES)]
    n_rngs = [(int(ncum[gb[c]]), int(ncum[gb[c + 1]])) for c in range(N_CORES)]

    BE = BATCH * GROUP
    max_e = max(e1 - e0 for e0, e1 in e_rngs)
    max_n = max(n1 - n0 for n0, n1 in n_rngs)
    EPC = -(-max_e // BE) * BE
    NPC = -(-max_n // NODE_GRAN) * NODE_GRAN

    ow = out_w.reshape(-1).astype(np.float64)
    absow = np.abs(ow)
    sgn = np.sign(ow)

    # Wf: col j = i*10 + o, scaled by |ow_o|; cols 750:768 zero
    j_i = np.arange(F_ATOM * OUT)
    Wcols = lin_w.astype(np.float64) * absow[j_i % OUT, None]      # [750,12]
    bcols = lin_b.astype(np.float64) * absow[j_i % OUT]
    Wf = np.zeros((F_BOND + 1, COLS_PAD), dtype=np.float32)
    Wf[:F_BOND, :F_ATOM * OUT] = Wcols.T
    Wf[F_BOND, :F_ATOM * OUT] = bcols
    Wf = _bf16(Wf)

    # Ssel: [128, 6*75]; chunk c at cols [c*75,(c+1)*75): row r, col i
    Ss = np.zeros((P, 6 * F_ATOM), dtype=np.float32)
    for c in range(6):
        j = c * P + np.arange(P)
        valid = j < F_ATOM * OUT
        jv = j[valid]
        Ss[np.arange(P)[valid], c * F_ATOM + jv // OUT] = sgn[jv % OUT]
    Ss = _bf16(Ss)
    ones = _bf16(np.ones((F_ATOM, 1), dtype=np.float32))

    rw1 = (root_w.astype(np.float64) @ ow).astype(np.float32)
    rw1_rep = np.ascontiguousarray(np.broadcast_to(rw1[None, :], (P, F_ATOM)),
                                   dtype=np.float32)

    in_maps = []
    for c in range(N_CORES):
        e0, e1 = e_rngs[c]
        ne = e1 - e0
        eaT = np.zeros((F_BOND + 1, EPC), dtype=np.float32)
        eaT[:F_BOND, :ne] = ea_s[e0:e1].T
        eaT[F_BOND, :ne] = 1.0
        xTc = np.zeros((F_ATOM, EPC), dtype=np.float32)
        xTc[:, :ne] = x[src_s[e0:e1]].T
        n0, n1 = n_rngs[c]
        nn_ = n1 - n0
        xsf = np.zeros((NPC, F_ATOM), dtype=np.float32)
        xsf[:nn_] = x[n0:n1]
        if v2:
            in_maps.append({
                "eaT": _bf16(eaT), "xT": _bf16(xTc), "Wf": Wf, "Ssel": Ss,
                "ones": ones, "xsT": _bf16(np.ascontiguousarray(xsf.T)),
                "rw1": _bf16(rw1[:, None]),
            })
        else:
            in_maps.append({
                "eaT": _bf16(eaT), "xT": _bf16(xTc), "Wf": Wf, "Ssel": Ss,
                "ones": ones, "xs": xsf, "rw1": rw1_rep,
            })

    cb1 = float(np.dot(conv_b.astype(np.float64), ow))
    ob = float(np.asarray(out_b).reshape(-1)[0])
    meta = dict(gb=gb, e_rngs=e_rngs, n_rngs=n_rngs, ge_s=ge_s, batch=batch,
                ncnt=ncnt, cb1=cb1, ob=ob, EPC=EPC, NPC=NPC)
    return in_maps, meta


def _bf16(a):
    import jax.numpy as jnp
    return np.asarray(jnp.asarray(a, dtype=jnp.bfloat16))


def _combine_v1(results, meta, G):
    gb = meta["gb"]
    out = np.zeros(G, dtype=np.float64)
    for c in range(N_CORES):
        g0, g1 = int(gb[c]), int(gb[c + 1])
        e0, e1 = meta["e_rngs"][c]
        n0, n1 = meta["n_rngs"][c]
        m1 = results[c]["m1o"].reshape(-1)[:e1 - e0]
        hxa = results[c]["hxo"]
        if hxa.ndim == 3:
            hx = hxa.transpose(0, 2, 1).reshape(-1)[:n1 - n0]
        else:
            hx = hxa.reshape(-1)[:n1 - n0]
        esum = np.bincount(meta["ge_s"][e0:e1] - g0, weights=m1,
                           minlength=g1 - g0)
        nsum = np.bincount(meta["batch"][n0:n1] - g0, weights=hx,
                           minlength=g1 - g0)
        out[g0:g1] = esum + nsum
    out += meta["ncnt"] * meta["cb1"] + meta["ob"]
    return out.astype(np.float32)[:, None]


def _combine(results, meta, G):
    gb = meta["gb"]
    out = np.zeros(G, dtype=np.float64)
    for c in range(N_CORES):
        g0, g1 = int(gb[c]), int(gb[c + 1])
        e0, e1 = meta["e_rngs"][c]
        n0, n1 = meta["n_rngs"][c]
        m1 = results[c]["m1o"].transpose(0, 2, 1).reshape(-1)[:e1 - e0]
        hx = results[c]["hxo"].transpose(0, 2, 1).reshape(-1)[:n1 - n0]
        esum = np.bincount(meta["ge_s"][e0:e1] - g0, weights=m1,
                           minlength=g1 - g0)
        nsum = np.bincount(meta["batch"][n0:n1] - g0, weights=hx,
                           minlength=g1 - g0)
        out[g0:g1] = esum + nsum
    out += meta["ncnt"] * meta["cb1"] + meta["ob"]
    return out.astype(np.float32)[:, None]


_EXEC_CACHE = {}


def _get_executor(nc):
    """Build a cached, re-runnable sharded executor for the Bass program
    (modeled on bass2jax.run_bass_via_pjrt, without buffer donation)."""
    import jax
    from jax.sharding import Mesh, PartitionSpec, NamedSharding
    from jax.experimental.shard_map import shard_map
    from concourse import bass2jax
    import concourse.mybir as mybir

    bass2jax.install_neuronx_cc_hook()
    partition_name = (nc.partition_id_tensor.name
                      if nc.partition_id_tensor else None)
    in_names, out_names, out_avals, zero_outs = [], [], [], []
    for alloc in nc.m.functions[0].allocations:
        if not isinstance(alloc, mybir.MemoryLocationSet):
            continue
        name = alloc.memorylocations[0].name
        if alloc.kind == "ExternalInput":
            if name != partition_name:
                in_names.append(name)
        elif alloc.kind == "ExternalOutput":
            out_names.append(name)
            shape = tuple(alloc.tensor_shape)
            dtype = mybir.dt.np(alloc.dtype)
            out_avals.append(jax.core.ShapedArray(shape, dtype))
            zero_outs.append(np.zeros(shape, dtype))
    n_params = len(in_names)
    all_in = in_names + out_names
    if partition_name is not None:
        all_in.append(partition_name)

    def _body(*args):
        operands = list(args)
        if partition_name is not None:
            operands.append(bass2jax.partition_id_tensor())
        outs = bass2jax._bass_exec_p.bind(
            *operands,
            out_avals=tuple(out_avals),
            in_names=tuple(all_in),
            out_names=tuple(out_names),
            lowering_input_output_aliases=(),
            sim_require_finite=True,
            sim_require_nnan=True,
            nc=nc,
        )
        return tuple(outs)

    devices = jax.devices()[:N_CORES]
    mesh = Mesh(np.asarray(devices), ("core",))
    n_outs = len(out_names)
    donate = tuple(range(n_params, n_params + n_outs))
    sharded = jax.jit(
        shard_map(_body, mesh=mesh,
                  in_specs=(PartitionSpec("core"),) * (n_params + n_outs),
                  out_specs=(PartitionSpec("core"),) * n_outs,
                  check_rep=False),
        donate_argnums=donate, keep_unused=True)
    sharding = NamedSharding(mesh, PartitionSpec("core"))
    return dict(fn=sharded, in_names=in_names, out_names=out_names,
                out_avals=out_avals, zero_outs=zero_outs, sharding=sharding)


def _run_spmd(nc, in_maps, bench_iters=0):
    """Run the program on 8 cores; returns (results, best_wall_ns)."""
    import time as _time
    import jax

    key = id(nc)
    if key not in _EXEC_CACHE:
        _EXEC_CACHE[key] = _get_executor(nc)
    ex = _EXEC_CACHE[key]

    concat_in = [
        np.concatenate([np.asarray(m[name]) for m in in_maps], axis=0)
        for name in ex["in_names"]
    ]
    dev_in = [jax.device_put(a, ex["sharding"]) for a in concat_in]

    def zeros():
        return [jax.device_put(
            np.zeros((N_CORES * z.shape[0], *z.shape[1:]), z.dtype),
            ex["sharding"]) for z in ex["zero_outs"]]

    out_arrs = ex["fn"](*dev_in, *zeros())
    jax.block_until_ready(out_arrs)
    best = None
    for _ in range(bench_iters):
        z = zeros()
        jax.block_until_ready(z)
        t0 = _time.perf_counter()
        o = ex["fn"](*dev_in, *z)
        jax.block_until_ready(o)
        dt = (_time.perf_counter() - t0) * 1e9
        best = dt if best is None else min(best, dt)
    results = [
        {name: np.asarray(out_arrs[i]).reshape(
            N_CORES, *ex["out_avals"][i].shape)[c]
         for i, name in enumerate(ex["out_names"])}
        for c in range(N_CORES)
    ]
    return results, best


VERSION = 2               # active kernel variant (0 flat, 1 T-layout, 2 tuned)
ACT_CHUNKS = (0, 1, 2, 3)  # which relu chunks go to ScalarE (rest on VectorE)


def _normalize(args):
    (x, edge_index, edge_attr, batch, lin_w, lin_b, root_w, conv_b,
     out_w, out_b, num_graphs) = args
    return (np.asarray(x, dtype=np.float32), np.asarray(edge_index),
            np.asarray(edge_attr, dtype=np.float32),
            np.asarray(batch).astype(np.int64),
            np.asarray(lin_w, dtype=np.float32),
            np.asarray(lin_b, dtype=np.float32),
            np.asarray(root_w, dtype=np.float32),
            np.asarray(conv_b, dtype=np.float32),
            np.asarray(out_w, dtype=np.float32),
            np.asarray(out_b, dtype=np.float32), int(num_graphs))


def _get_program(meta, version, repeat=1):
    if version == 2:
        key = (2, meta["EPC"] // (BATCH * GROUP), meta["NPC"] // P,
               ACT_CHUNKS, repeat)
        if key not in _PROG_CACHE:
            _PROG_CACHE[key] = _build_program_v2(key[1], key[2], ACT_CHUNKS,
                                                 repeat=repeat)
        return _PROG_CACHE[key]
    if version == 1:
        key = (1, meta["EPC"] // (BATCH * GROUP), meta["NPC"] // P,
               ACT_CHUNKS, repeat)
        if key not in _PROG_CACHE:
            _PROG_CACHE[key] = _build_program_v1(key[1], key[2], ACT_CHUNKS,
                                                 repeat=repeat)
    else:
        key = (0, meta["EPC"] // P, meta["NPC"] // P, meta["kp"], meta["kn"],
               repeat)
        if key not in _PROG_CACHE:
            _PROG_CACHE[key] = _build_program(key[1], key[2], key[3], key[4],
                                              repeat=repeat)
    return _PROG_CACHE[key]


def _run_version(np_args, version, repeat=1, bench_iters=0):
    G = np_args[-1]
    if version == 2:
        in_maps, meta = _prep_v1(*np_args[:-1], G, v2=True)
    elif version == 1:
        in_maps, meta = _prep_v1(*np_args[:-1], G)
    else:
        in_maps, meta = _prep(*np_args[:-1], G)
    nc = _get_program(meta, version, repeat)
    results, best_ns = _run_spmd(nc, in_maps, bench_iters=bench_iters)
    out = (_combine_v1 if version >= 1 else _combine)(results, meta, G)
    return out, best_ns


def bench(np_inputs, version=None, R=128, iters=8):
    """Returns (hw_ns_estimate, wall1_ns, wallR_ns) using an in-program
    repeat loop to amortize the RPC floor."""
    version = VERSION if version is None else version
    np_args = _normalize(np_inputs)
    out1, w1 = _run_version(np_args, version, repeat=1, bench_iters=iters)
    outR, wR = _run_version(np_args, version, repeat=R, bench_iters=iters)
    assert np.allclose(out1, outR, rtol=1e-3, atol=1e-5), "repeat mismatch"
    hw = (wR - w1) / (R - 1)
    return hw, w1, wR, out1


def kernel(x, edge_index, edge_attr, batch, lin_w, lin_b, root_w, conv_b,
           out_w, out_b, num_graphs, _bench_iters=0):
    np_args = _normalize((x, edge_index, edge_attr, batch, lin_w, lin_b,
                          root_w, conv_b, out_w, out_b, num_graphs))
    out, best_ns = _run_version(np_args, VERSION, repeat=1,
                                bench_iters=_bench_iters)
    if _bench_iters:
        return out, best_ns
    return out
